# revision 29
# baseline (speedup 1.0000x reference)
"""Trainium2 Bass kernel: dense transformer block (attention + per-batch bmm + FF).

Sharding: 8 cores = (batch b = c//2) x (query-half nh = c%2).
Each core computes attention for all 16 heads over its 1024 query rows
(keys/values over full S=2048, recomputed per batch-pair), then the
per-batch feature-reduction bmm and the feed-forward for its rows.

Engine balance (cost model: time = free-size x cycle, partitions free):
 - PE does two passes over scores (pass-1 raw for the row max, pass-2 with a
   65th contraction row k_aug=-1 / q_aug=+rowmax so ScalarE exps straight out
   of PSUM).  All matmuls fp32r/bf16 at 1 cycle/row.
 - The pass-1 running-max scan is split across engines: head0's chain on
   VectorE, head1's chain on GPSIMD, so neither engine becomes the
   bottleneck (it was 337us on DVE alone).
 - Software pipeline: pass-2+AV of iteration it-2 is interleaved one
   matmul-group at a time into pass-1 of iteration it, keeping PE dense.
 - Odd heads live on partitions 64:128 (aug row at 63, ones-first V column
   order) so every PSUM evacuation is lane-aligned and runs on ScalarE.
 - Softmax denominator comes from a ones-column in V; normalization is
   reciprocal (DVE) + gpsimd partition_broadcast + one tensor_tensor.
"""

import sys

sys.path.insert(0, "/opt/trn_rl_repo")

import numpy as np

B, S, E, H, HF = 4, 2048, 1024, 16, 64
NH = 1024          # query rows per core
SCALE = 1.0 / np.sqrt(HF)
NIT = 16           # (head-pair, query-block) iterations: 8 hp x 2 nb

_CACHE = {}
_CST = np.concatenate([np.ones((1, S), np.float32), -np.ones((1, S), np.float32)])


def _np_reference(x, attention_mask, Wq, Wk, Wv, Wr, Wff, bff):
    """Fallback (used only if the mask is not all-ones)."""
    x64 = x.astype(np.float64)
    q = np.einsum("bse,hef->bhsf", x64, Wq.astype(np.float64)).reshape(B * H, S, HF)
    k = np.einsum("bse,hef->bhsf", x64, Wk.astype(np.float64)).reshape(B * H, S, HF)
    v = np.einsum("bse,hef->bhsf", x64, Wv.astype(np.float64)).reshape(B * H, S, HF)
    s = np.matmul(q, k.transpose(0, 2, 1))
    s = np.where(attention_mask[0] == 0, -1e9, s)
    s = s * SCALE
    s = s - s.max(axis=-1, keepdims=True)
    p = np.exp(s)
    p /= p.sum(axis=-1, keepdims=True)
    z = np.matmul(p, v).reshape(B, H, S, HF).transpose(0, 2, 1, 3).reshape(B, S, E)
    z = np.matmul(z, Wr.astype(np.float64))
    o = np.maximum(z @ Wff.astype(np.float64).T + bff.astype(np.float64), 0.0)
    return o.astype(np.float32)


def _build():
    import concourse.bacc as bacc
    import concourse.bass as bass
    import concourse.mybir as mybir
    import concourse.tile as tile
    import bass_rust

    F32 = mybir.dt.float32
    BF16 = mybir.dt.bfloat16
    F32R = mybir.dt.float32r
    MULT = mybir.AluOpType.mult
    MAXOP = mybir.AluOpType.max
    EXP = mybir.ActivationFunctionType.Exp
    RELU = mybir.ActivationFunctionType.Relu
    RMAX = bass_rust.ReduceOp.max
    PSUM = bass.MemorySpace.PSUM

    def r(ap):
        return ap.bitcast(F32R)

    nc = bacc.Bacc("TRN2", target_bir_lowering=False, debug=False)
    xt_d = nc.dram_tensor("xt", [E, S], F32, kind="ExternalInput")      # x[b].T, cols rolled so my half is first
    wq_d = nc.dram_tensor("wq", [E, E], F32, kind="ExternalInput")      # [e, h*HF+f]
    wk_d = nc.dram_tensor("wk", [E, E], F32, kind="ExternalInput")
    wv_d = nc.dram_tensor("wv", [E, E], F32, kind="ExternalInput")
    wr_d = nc.dram_tensor("wr", [E, E], F32, kind="ExternalInput")      # Wr[b]
    wfft_d = nc.dram_tensor("wfft", [E, E], F32, kind="ExternalInput")  # Wff.T
    bff_d = nc.dram_tensor("bff", [E, 1], F32, kind="ExternalInput")
    cst_d = nc.dram_tensor("cst", [2, S], F32, kind="ExternalInput")    # rows: +1.0, -1.0
    out_d = nc.dram_tensor("o", [E, NH], F32, kind="ExternalOutput")    # [j, n]

    with tile.TileContext(nc) as tc:
        with tc.tile_pool(name="glob", bufs=1) as glob:
            zTn = glob.tile([128, 8, NH], F32)     # normalized z^T: [f-in-pair, echunk, n]
            bfft = glob.tile([128, 8], F32)
            ones65 = glob.tile([65, 65], F32)      # ones row at partition 64 for PE broadcast
            nc.vector.memset(ones65, 1.0)

            # ---------------- phase 1: projections + attention ----------------
            with tc.tile_pool(name="p1x", bufs=1) as p1x, \
                 tc.tile_pool(name="wqk", bufs=2) as wqk, \
                 tc.tile_pool(name="wvp", bufs=1) as wvp, \
                 tc.tile_pool(name="qkpool", bufs=2) as qkpool, \
                 tc.tile_pool(name="vpool", bufs=2) as vpool, \
                 tc.tile_pool(name="epool", bufs=4) as epool, \
                 tc.tile_pool(name="spool", bufs=1) as spool, \
                 tc.tile_pool(name="ps_a", bufs=2, space=PSUM) as ps_a, \
                 tc.tile_pool(name="ps_2", bufs=2, space=PSUM) as ps_2, \
                 tc.tile_pool(name="ps_z", bufs=2, space=PSUM) as ps_z:

                wst = {}     # prefetched weight tiles per hp
                qk = {}      # (qa0, qa1, ka0, ka1) per hp
                v4g = {}     # v tiles per group g = hp//2
                macc = {}    # [128,2,512] running max per it (both heads)
                zps = {}     # (it, h) -> PSUM z accumulator
                expt = {}    # (it, h, mt) -> exp'd score tile
                norm_q = []  # deferred normalization closures

                def prefetch_w(hp):
                    wq_sb = wqk.tile([128, 8, 128], F32, tag="wq")
                    nc.sync.dma_start(out=r(wq_sb), in_=r(wq_d.ap()).rearrange("(i p) c -> p i c", p=128)[:, :, hp * 128:(hp + 1) * 128])
                    wk_sb = wqk.tile([128, 8, 128], F32, tag="wk")
                    nc.sync.dma_start(out=r(wk_sb), in_=r(wk_d.ap()).rearrange("(i p) c -> p i c", p=128)[:, :, hp * 128:(hp + 1) * 128])
                    wv_sb = None
                    if hp % 2 == 0:
                        g = hp // 2
                        wv_sb = wvp.tile([128, 8, 256], F32, tag="wv")
                        nc.sync.dma_start(out=r(wv_sb), in_=r(wv_d.ap()).rearrange("(i p) c -> p i c", p=128)[:, :, g * 256:(g + 1) * 256])
                    wst[hp] = (wq_sb, wk_sb, wv_sb)

                # startup: weights for hp0 first, then x in column chunks
                prefetch_w(0)
                xt = p1x.tile([128, 8, S], F32)    # 64KB/part
                for c in range(4):
                    nc.sync.dma_start(
                        out=r(xt[:, :, c * 512:(c + 1) * 512]),
                        in_=r(xt_d.ap()).rearrange("(i p) m -> p i m", p=128)[:, :, c * 512:(c + 1) * 512])

                def evac_qk(pair, half, dst0, dst1, dsl):
                    # even head rows 0:64 -> dst0 (Act), odd head rows 64:128 staged (DVE) + DMA shift
                    nc.scalar.copy(out=r(dst0[0:64, dsl]), in_=pair[0:64, half, :])
                    st = spool.tile([128, 512], F32, tag="stg", name="stg", bufs=2)
                    nc.vector.tensor_copy(out=st[64:128, :], in_=pair[64:128, half, :])
                    nc.scalar.dma_start(out=r(dst1[0:64, dsl]), in_=r(st[64:128, :]))

                def proj(hp):
                    wq_sb, wk_sb, wv_sb = wst.pop(hp)
                    qa0 = qkpool.tile([65, NH], F32, tag="qa0")
                    qa1 = qkpool.tile([65, NH], F32, tag="qa1")
                    ka0 = qkpool.tile([65, S], F32, tag="ka0")
                    ka1 = qkpool.tile([65, S], F32, tag="ka1")
                    qk[hp] = (qa0, qa1, ka0, ka1)
                    nc.sync.dma_start(out=r(ka0[64:65, :]), in_=r(cst_d.ap())[1:2, :])
                    nc.sync.dma_start(out=r(ka1[64:65, :]), in_=r(cst_d.ap())[1:2, :])

                    # q/k in pair tiles: [q(c)|k(c)] for c<2, then [k2|k3]
                    for c in range(3):
                        pr = ps_a.tile([128, 2, 512], F32, tag="pair", name="projpair")
                        lhs0 = wq_sb if c < 2 else wk_sb
                        sl0 = slice(c * 512, (c + 1) * 512) if c < 2 else slice(2 * 512, 3 * 512)
                        for e in range(8):
                            nc.tensor.matmul(pr[:, 0, :], r(lhs0[:, e, :]), r(xt[:, e, sl0]),
                                             start=(e == 0), stop=(e == 7))
                        sl1 = slice(c * 512, (c + 1) * 512) if c < 2 else slice(3 * 512, 4 * 512)
                        for e in range(8):
                            nc.tensor.matmul(pr[:, 1, :], r(wk_sb[:, e, :]), r(xt[:, e, sl1]),
                                             start=(e == 0), stop=(e == 7))
                        if c < 2:
                            evac_qk(pr, 0, qa0, qa1, sl0)
                        else:
                            evac_qk(pr, 0, ka0, ka1, sl0)
                        evac_qk(pr, 1, ka0, ka1, sl1)

                    if wv_sb is not None:          # v for 4 heads, two m-tiles per pair tile
                        g = hp // 2
                        v4 = vpool.tile([128, 16, 4, 65], BF16, tag="v4")
                        v4g[g] = v4
                        nc.vector.memset(v4[:, :, :, 64:65], 1.0)
                        for mp in range(8):
                            pr = ps_a.tile([128, 2, 512], F32, tag="pair", name="vpair")
                            for t in range(2):
                                mt = 2 * mp + t
                                for e in range(8):
                                    nc.tensor.matmul(pr[:, t, 0:256], r(xt[:, e, mt * 128:(mt + 1) * 128]), r(wv_sb[:, e, :]),
                                                     start=(e == 0), stop=(e == 7))
                            nc.scalar.copy(out=v4[:, 2 * mp:2 * mp + 2, :, 0:64],
                                           in_=pr[:, :, 0:256].rearrange("p m (s f) -> p m s f", s=4))

                def p1_step(it, mt):
                    hp, nb = it // 2, it % 2
                    qa0, qa1, ka0, ka1 = qk[hp]
                    nsl = slice(nb * 512, (nb + 1) * 512)
                    msl = slice(mt * 128, (mt + 1) * 128)
                    pr = ps_a.tile([128, 2, 512], F32, tag="pair", name="ps1pair")
                    nc.tensor.matmul(pr[:, 0, :], r(ka0[0:64, msl]), r(qa0[0:64, nsl]), start=True, stop=True)
                    nc.tensor.matmul(pr[:, 1, :], r(ka1[0:64, msl]), r(qa1[0:64, nsl]), start=True, stop=True)
                    if mt == 0:
                        m2 = spool.tile([128, 2, 512], BF16, tag="macc2", name="m2")
                        macc[it] = m2
                        nc.vector.tensor_copy(out=m2, in_=pr)
                    else:
                        m2 = macc[it]
                        nc.vector.tensor_tensor(out=m2, in0=pr, in1=m2, op=MAXOP)

                def p1_finish(it):
                    hp, nb = it // 2, it % 2
                    qa0, qa1, _, _ = qk[hp]
                    nsl = slice(nb * 512, (nb + 1) * 512)
                    m2 = macc.pop(it)
                    mall0 = spool.tile([128, 512], F32, tag="mall")
                    nc.gpsimd.partition_all_reduce(mall0, m2[:, 0, :], channels=128, reduce_op=RMAX)
                    nc.scalar.dma_start(out=r(qa0[64:65, nsl]), in_=r(mall0[0:1, :]))
                    mall1 = spool.tile([128, 512], F32, tag="mall")
                    nc.gpsimd.partition_all_reduce(mall1, m2[:, 1, :], channels=128, reduce_op=RMAX)
                    nc.scalar.dma_start(out=r(qa1[64:65, nsl]), in_=r(mall1[0:1, :]))

                def emit_av(it, h, j):
                    hp = it // 2
                    hh = 2 * hp + h
                    g, s4 = hh // 4, hh % 4
                    v4 = v4g[g]
                    zp = zps[(it, h)]
                    for t in range(2):
                        mt = 2 * j + t
                        et = expt.pop((it, h, mt))
                        nc.tensor.matmul(zp[0:65, :], v4[:, mt, s4, 0:65], et,
                                         start=(j == 0 and t == 0), stop=(j == 7 and t == 1))

                def p2av_step(it, step):
                    hp, nb = it // 2, it % 2
                    qa0, qa1, ka0, ka1 = qk[hp]
                    nsl = slice(nb * 512, (nb + 1) * 512)
                    h, j = step % 2, step // 2
                    if j == 0:
                        zps[(it, h)] = ps_z.tile([128, 512], F32, tag="zps", name="zps")
                    if j >= 1:
                        emit_av(it, h, j - 1)
                    for t in range(2):
                        mt = 2 * j + t
                        msl = slice(mt * 128, (mt + 1) * 128)
                        ka = ka0 if h == 0 else ka1
                        qa = qa0 if h == 0 else qa1
                        ps2t = ps_2.tile([128, 512], F32, tag="ps2", name="ps2t")
                        nc.tensor.matmul(ps2t, r(ka[0:65, msl]), r(qa[0:65, nsl]), start=True, stop=True)
                        et = epool.tile([128, 512], BF16, tag="expt", name="et")
                        nc.scalar.activation(out=et, in_=ps2t, func=EXP, scale=float(SCALE))
                        expt[(it, h, mt)] = et

                def p2av_finish(it):
                    hp, nb = it // 2, it % 2
                    nsl = slice(nb * 512, (nb + 1) * 512)
                    emit_av(it, 0, 7)
                    emit_av(it, 1, 7)
                    zp0 = zps.pop((it, 0))
                    zp1 = zps.pop((it, 1))
                    st = {}

                    def n0():
                        st["rec0"] = rec0 = spool.tile([65, 512], F32, tag="rec0", name="rec0")
                        nc.vector.reciprocal(out=rec0[64:65, :], in_=zp0[64:65, :])
                        st["recS0"] = recS0 = spool.tile([1, 512], F32, tag="recS", name="recS0")
                        nc.scalar.dma_start(out=recS0, in_=rec0[64:65, :])

                    def n1():
                        st["recb0"] = recb0 = spool.tile([64, 512], F32, tag="recb0", name="recb0")
                        nc.gpsimd.partition_broadcast(out_ap=recb0[0:64, :], in_ap=st["recS0"][0:1, :], channels=64)
                        nc.vector.tensor_tensor(out=r(zTn[0:64, hp, nsl]), in0=zp0[0:64, :], in1=recb0[0:64, :], op=MULT)

                    def n2():
                        st["rec1"] = rec1 = spool.tile([65, 512], F32, tag="rec1", name="rec1")
                        nc.vector.reciprocal(out=rec1[64:65, :], in_=zp1[64:65, :])
                        st["recS1"] = recS1 = spool.tile([1, 512], F32, tag="recS", name="recS1")
                        nc.scalar.dma_start(out=recS1, in_=rec1[64:65, :])

                    def n3():
                        st["recb1"] = recb1 = spool.tile([64, 512], F32, tag="recb1", name="recb1")
                        nc.gpsimd.partition_broadcast(out_ap=recb1[0:64, :], in_ap=st["recS1"][0:1, :], channels=64)
                        nc.vector.tensor_tensor(out=recb1[0:64, :], in0=zp1[0:64, :], in1=recb1[0:64, :], op=MULT)
                        nc.scalar.dma_start(out=r(zTn[64:128, hp, nsl]), in_=r(recb1[0:64, :]))

                    norm_q.extend([n0, n1, n2, n3])

                for it in range(NIT + 2):
                    hp, nb = it // 2, it % 2
                    while norm_q:
                        norm_q.pop(0)()
                    if it < NIT and nb == 1 and hp + 1 < 8:
                        prefetch_w(hp + 1)         # weights for next hp land during this it
                    if it < NIT and nb == 0:
                        proj(hp)
                    prev = it - 2
                    for step in range(16):
                        if 0 <= prev:
                            p2av_step(prev, step)
                        if it < NIT:
                            p1_step(it, step)
                    if 0 <= prev:
                        p2av_finish(prev)
                    if it < NIT:
                        p1_finish(it)
                while norm_q:
                    norm_q.pop(0)()
                # drop remaining refs so pools can close
                qk.clear(); v4g.clear(); wst.clear()

            # ---------------- phase 2: y^T = Wr bmm, then FF ----------------
            with tc.tile_pool(name="p2w", bufs=1) as p2w, \
                 tc.tile_pool(name="p2y", bufs=1) as p2y, \
                 tc.tile_pool(name="p2o", bufs=2) as p2o, \
                 tc.tile_pool(name="ps_y", bufs=2, space=PSUM) as ps_y:
                wr_sb = p2w.tile([128, 8, 8, 128], F32, tag="wr")   # [p, ec, dt, d]
                for dt in range(8):
                    nc.sync.dma_start(
                        out=r(wr_sb[:, :, dt, :]),
                        in_=r(wr_d.ap()).rearrange("(i p) c -> p i c", p=128)[:, :, dt * 128:(dt + 1) * 128])
                wff_sb = p2w.tile([128, 8, E], F32, tag="wff")
                nc.sync.dma_start(out=r(wff_sb), in_=r(wfft_d.ap()).rearrange("(i p) c -> p i c", p=128))
                nc.sync.dma_start(out=bfft, in_=bff_d.ap().rearrange("(t p) o -> p (t o)", p=128))
                yT = p2y.tile([128, 8, NH], F32)
                for dt in range(8):
                    psy = ps_y.tile([128, NH], F32, tag="psy")
                    for ec in range(8):
                        for half in range(2):
                            nc.tensor.matmul(psy[:, half * 512:(half + 1) * 512],
                                             r(wr_sb[:, ec, dt, :]),
                                             r(zTn[:, ec, half * 512:(half + 1) * 512]),
                                             start=(ec == 0), stop=(ec == 7))
                    nc.scalar.copy(out=r(yT[:, dt, :]), in_=psy)
                for jt in range(8):
                    pso = ps_y.tile([128, NH], F32, tag="psy")
                    for dc in range(8):
                        for half in range(2):
                            nc.tensor.matmul(pso[:, half * 512:(half + 1) * 512],
                                             r(wff_sb[:, dc, jt * 128:(jt + 1) * 128]),
                                             r(yT[:, dc, half * 512:(half + 1) * 512]),
                                             start=(dc == 0), stop=(dc == 7))
                    ot = p2o.tile([128, NH], F32, tag="ot")
                    nc.scalar.activation(out=ot, in_=pso, func=RELU, bias=bfft[:, jt:jt + 1], scale=1.0)
                    nc.sync.dma_start(out=out_d.ap()[jt * 128:(jt + 1) * 128, :], in_=ot)

    nc.compile()
    return nc


def _get_module():
    if "nc" not in _CACHE:
        _CACHE["nc"] = _build()
    return _CACHE["nc"]


def kernel(x, attention_mask, Wq, Wk, Wv, Wr, Wff, bff):
    from concourse import bass_utils

    x = np.asarray(x, dtype=np.float32)
    attention_mask = np.asarray(attention_mask)
    Wq = np.asarray(Wq, dtype=np.float32)
    Wk = np.asarray(Wk, dtype=np.float32)
    Wv = np.asarray(Wv, dtype=np.float32)
    Wr = np.asarray(Wr, dtype=np.float32)
    Wff = np.asarray(Wff, dtype=np.float32)
    bff = np.asarray(bff, dtype=np.float32)

    if not np.all(attention_mask == 1):
        return _np_reference(x, attention_mask, Wq, Wk, Wv, Wr, Wff, bff)

    nc = _get_module()
    wq2 = np.ascontiguousarray(Wq.transpose(1, 0, 2).reshape(E, E))
    wk2 = np.ascontiguousarray(Wk.transpose(1, 0, 2).reshape(E, E))
    wv2 = np.ascontiguousarray(Wv.transpose(1, 0, 2).reshape(E, E))
    wfft = np.ascontiguousarray(Wff.T)
    bff2 = np.ascontiguousarray(bff.reshape(E, 1))

    in_maps = []
    for c in range(8):
        b, nh = c // 2, c % 2
        xt = x[b].T
        if nh:
            xt = np.concatenate([xt[:, NH:], xt[:, :NH]], axis=1)
        in_maps.append({
            "xt": np.ascontiguousarray(xt),
            "wq": wq2, "wk": wk2, "wv": wv2,
            "wr": np.ascontiguousarray(Wr[b]),
            "wfft": wfft, "bff": bff2,
            "cst": _CST,
        })

    res = bass_utils.run_bass_kernel_spmd(nc, in_maps, core_ids=list(range(8)), **_CACHE.get("run_kwargs", {}))
    _CACHE["last_result"] = res

    out = np.empty((B, S, E), dtype=np.float32)
    for c in range(8):
        b, nh = c // 2, c % 2
        out[b, nh * NH:(nh + 1) * NH, :] = res.results[c]["o"].T
    return out


# revision 31
# speedup vs baseline: 1.0442x; 1.0442x over previous
"""Trainium2 Bass kernel: dense transformer block (attention + per-batch bmm + FF).

Sharding: 8 cores = (batch b = c//2) x (query-half nh = c%2).
Each core computes attention for all 16 heads over its 1024 query rows
(keys/values over full S=2048, recomputed per batch-pair), then the
per-batch feature-reduction bmm and the feed-forward for its rows.

Engine balance (cost model: time = free-size x cycle, partitions free):
 - PE does two passes over scores (pass-1 raw for the row max, pass-2 with a
   65th contraction row k_aug=-1 / q_aug=+rowmax so ScalarE exps straight out
   of PSUM).  All matmuls fp32r/bf16 at 1 cycle/row.
 - The pass-1 running-max scan is split across engines: head0's chain on
   VectorE, head1's chain on GPSIMD, so neither engine becomes the
   bottleneck (it was 337us on DVE alone).
 - Software pipeline: pass-2+AV of iteration it-2 is interleaved one
   matmul-group at a time into pass-1 of iteration it, keeping PE dense.
 - Odd heads live on partitions 64:128 (aug row at 63, ones-first V column
   order) so every PSUM evacuation is lane-aligned and runs on ScalarE.
 - Softmax denominator comes from a ones-column in V; normalization is
   reciprocal (DVE) + gpsimd partition_broadcast + one tensor_tensor.
"""

import sys

sys.path.insert(0, "/opt/trn_rl_repo")

import numpy as np

B, S, E, H, HF = 4, 2048, 1024, 16, 64
NH = 1024          # query rows per core
SCALE = 1.0 / np.sqrt(HF)
NIT = 16           # (head-pair, query-block) iterations: 8 hp x 2 nb

_CACHE = {}
_CST = np.concatenate([np.ones((1, S), np.float32), -np.ones((1, S), np.float32)])


def _np_reference(x, attention_mask, Wq, Wk, Wv, Wr, Wff, bff):
    """Fallback (used only if the mask is not all-ones)."""
    x64 = x.astype(np.float64)
    q = np.einsum("bse,hef->bhsf", x64, Wq.astype(np.float64)).reshape(B * H, S, HF)
    k = np.einsum("bse,hef->bhsf", x64, Wk.astype(np.float64)).reshape(B * H, S, HF)
    v = np.einsum("bse,hef->bhsf", x64, Wv.astype(np.float64)).reshape(B * H, S, HF)
    s = np.matmul(q, k.transpose(0, 2, 1))
    s = np.where(attention_mask[0] == 0, -1e9, s)
    s = s * SCALE
    s = s - s.max(axis=-1, keepdims=True)
    p = np.exp(s)
    p /= p.sum(axis=-1, keepdims=True)
    z = np.matmul(p, v).reshape(B, H, S, HF).transpose(0, 2, 1, 3).reshape(B, S, E)
    z = np.matmul(z, Wr.astype(np.float64))
    o = np.maximum(z @ Wff.astype(np.float64).T + bff.astype(np.float64), 0.0)
    return o.astype(np.float32)


def _build():
    import concourse.bacc as bacc
    import concourse.bass as bass
    import concourse.mybir as mybir
    import concourse.tile as tile
    import bass_rust

    F32 = mybir.dt.float32
    BF16 = mybir.dt.bfloat16
    F32R = mybir.dt.float32r
    MULT = mybir.AluOpType.mult
    MAXOP = mybir.AluOpType.max
    EXP = mybir.ActivationFunctionType.Exp
    RELU = mybir.ActivationFunctionType.Relu
    RMAX = bass_rust.ReduceOp.max
    PSUM = bass.MemorySpace.PSUM

    def r(ap):
        return ap.bitcast(F32R)

    nc = bacc.Bacc("TRN2", target_bir_lowering=False, debug=False)
    xt_d = nc.dram_tensor("xt", [E, S], F32, kind="ExternalInput")      # x[b].T, cols rolled so my half is first
    wq_d = nc.dram_tensor("wq", [E, E], F32, kind="ExternalInput")      # [e, h*HF+f]
    wk_d = nc.dram_tensor("wk", [E, E], F32, kind="ExternalInput")
    wv_d = nc.dram_tensor("wv", [E, E], F32, kind="ExternalInput")
    wr_d = nc.dram_tensor("wr", [E, E], F32, kind="ExternalInput")      # Wr[b]
    wfft_d = nc.dram_tensor("wfft", [E, E], F32, kind="ExternalInput")  # Wff.T
    bff_d = nc.dram_tensor("bff", [E, 1], F32, kind="ExternalInput")
    cst_d = nc.dram_tensor("cst", [2, S], F32, kind="ExternalInput")    # rows: +1.0, -1.0
    out_d = nc.dram_tensor("o", [E, NH], F32, kind="ExternalOutput")    # [j, n]

    with tile.TileContext(nc) as tc:
        with tc.tile_pool(name="glob", bufs=1) as glob:
            zTn = glob.tile([128, 8, NH], F32)     # normalized z^T: [f-in-pair, echunk, n]
            bfft = glob.tile([128, 8], F32)
            ones65 = glob.tile([65, 65], F32)      # ones row at partition 64 for PE broadcast
            nc.vector.memset(ones65, 1.0)

            # ---------------- phase 1: projections + attention ----------------
            with tc.tile_pool(name="p1x", bufs=1) as p1x, \
                 tc.tile_pool(name="wqk", bufs=2) as wqk, \
                 tc.tile_pool(name="wvp", bufs=1) as wvp, \
                 tc.tile_pool(name="qkpool", bufs=2) as qkpool, \
                 tc.tile_pool(name="vpool", bufs=2) as vpool, \
                 tc.tile_pool(name="epool", bufs=4) as epool, \
                 tc.tile_pool(name="spool", bufs=1) as spool, \
                 tc.tile_pool(name="ps_a", bufs=2, space=PSUM) as ps_a, \
                 tc.tile_pool(name="ps_2", bufs=2, space=PSUM) as ps_2, \
                 tc.tile_pool(name="ps_z", bufs=2, space=PSUM) as ps_z:

                wst = {}     # prefetched weight tiles per hp
                qk = {}      # (qa0, qa1, ka0, ka1) per hp
                v4g = {}     # v tiles per group g = hp//2
                macc = {}    # [128,2,512] running max per it (both heads)
                zps = {}     # (it, h) -> PSUM z accumulator
                expt = {}    # (it, h, mt) -> exp'd score tile
                norm_q = []  # deferred normalization closures

                def prefetch_w(hp):
                    wq_sb = wqk.tile([128, 8, 128], F32, tag="wq")
                    nc.sync.dma_start(out=r(wq_sb), in_=r(wq_d.ap()).rearrange("(i p) c -> p i c", p=128)[:, :, hp * 128:(hp + 1) * 128])
                    wk_sb = wqk.tile([128, 8, 128], F32, tag="wk")
                    nc.sync.dma_start(out=r(wk_sb), in_=r(wk_d.ap()).rearrange("(i p) c -> p i c", p=128)[:, :, hp * 128:(hp + 1) * 128])
                    wv_sb = None
                    if hp % 2 == 0:
                        g = hp // 2
                        wv_sb = wvp.tile([128, 8, 256], F32, tag="wv")
                        nc.sync.dma_start(out=r(wv_sb), in_=r(wv_d.ap()).rearrange("(i p) c -> p i c", p=128)[:, :, g * 256:(g + 1) * 256])
                    wst[hp] = (wq_sb, wk_sb, wv_sb)

                # startup: weights for hp0 first, then x in column chunks
                prefetch_w(0)
                xt = p1x.tile([128, 8, S], F32)    # 64KB/part
                for c in range(8):
                    nc.sync.dma_start(
                        out=r(xt[:, :, c * 256:(c + 1) * 256]),
                        in_=r(xt_d.ap()).rearrange("(i p) m -> p i m", p=128)[:, :, c * 256:(c + 1) * 256])

                def evac_qk(pair, half, dst0, dst1, dsl):
                    # even head rows 0:64 -> dst0 (Act), odd head rows 64:128 staged (DVE) + DMA shift
                    nc.scalar.copy(out=r(dst0[0:64, dsl]), in_=pair[0:64, half, :])
                    st = spool.tile([128, 512], F32, tag="stg", name="stg", bufs=2)
                    nc.vector.tensor_copy(out=st[64:128, :], in_=pair[64:128, half, :])
                    nc.scalar.dma_start(out=r(dst1[0:64, dsl]), in_=r(st[64:128, :]))

                projq = []   # per-step projection closures

                def queue_proj(hp, immediate=0):
                    """Allocate hp's qk tiles and queue its 6 half-closures."""
                    wq_sb, wk_sb, wv_sb = wst.pop(hp)
                    qa0 = qkpool.tile([65, NH], F32, tag="qa0", name="qa0")
                    qa1 = qkpool.tile([65, NH], F32, tag="qa1", name="qa1")
                    ka0 = qkpool.tile([65, S], F32, tag="ka0", name="ka0")
                    ka1 = qkpool.tile([65, S], F32, tag="ka1", name="ka1")
                    qk[hp] = (qa0, qa1, ka0, ka1)
                    nc.sync.dma_start(out=r(ka0[64:65, :]), in_=r(cst_d.ap())[1:2, :])
                    nc.sync.dma_start(out=r(ka1[64:65, :]), in_=r(cst_d.ap())[1:2, :])
                    pend = {}

                    def qk_half(kind, c):
                        def go():
                            if kind == "q" or (kind == "k" and c % 2 == 0):
                                pend["pr"] = ps_a.tile([128, 2, 512], F32, tag="pair", name="projpair")
                            pr = pend["pr"]
                            half = 0 if (kind == "q" or c == 2) else 1
                            w_sb = wq_sb if kind == "q" else wk_sb
                            sl = slice(c * 512, (c + 1) * 512)
                            for e in range(8):
                                nc.tensor.matmul(pr[:, half, :], r(w_sb[:, e, :]), r(xt[:, e, sl]),
                                                 start=(e == 0), stop=(e == 7))
                            if kind == "q":
                                evac_qk(pr, half, qa0, qa1, sl)
                            else:
                                evac_qk(pr, half, ka0, ka1, sl)
                        return go

                    # pair layout: [q(c)|k(c)] for c in 0,1 and [k2|k3]
                    halves = [qk_half("q", 0), qk_half("k", 0), qk_half("q", 1),
                              qk_half("k", 1), qk_half("k", 2), qk_half("k", 3)]
                    for f in halves[:immediate]:
                        f()
                    projq.extend(halves[immediate:])
                    return wv_sb

                def queue_vproj(g, wv_sb):
                    v4 = vpool.tile([128, 16, 4, 65], BF16, tag="v4", name="v4")
                    v4g[g] = v4
                    nc.vector.memset(v4[:, :, :, 64:65], 1.0)
                    pend = {}

                    def v_half(mt):
                        def go():
                            if mt % 2 == 0:
                                pend["pr"] = ps_a.tile([128, 2, 512], F32, tag="pair", name="vpair")
                            pr = pend["pr"]
                            t = mt % 2
                            for e in range(8):
                                nc.tensor.matmul(pr[:, t, 0:256], r(xt[:, e, mt * 128:(mt + 1) * 128]), r(wv_sb[:, e, :]),
                                                 start=(e == 0), stop=(e == 7))
                            if t == 1:
                                nc.scalar.copy(out=v4[:, mt - 1:mt + 1, :, 0:64],
                                               in_=pr[:, :, 0:256].rearrange("p m (s f) -> p m s f", s=4))
                        return go
                    projq.extend([v_half(mt) for mt in range(16)])

                def p1_step(it, mt):
                    hp, nb = it // 2, it % 2
                    qa0, qa1, ka0, ka1 = qk[hp]
                    nsl = slice(nb * 512, (nb + 1) * 512)
                    msl = slice(mt * 128, (mt + 1) * 128)
                    pr = ps_a.tile([128, 2, 512], F32, tag="pair", name="ps1pair")
                    nc.tensor.matmul(pr[:, 0, :], r(ka0[0:64, msl]), r(qa0[0:64, nsl]), start=True, stop=True)
                    nc.tensor.matmul(pr[:, 1, :], r(ka1[0:64, msl]), r(qa1[0:64, nsl]), start=True, stop=True)
                    if mt == 0:
                        m2 = spool.tile([128, 2, 512], BF16, tag="macc2", name="m2")
                        macc[it] = m2
                        nc.vector.tensor_copy(out=m2, in_=pr)
                    else:
                        m2 = macc[it]
                        nc.vector.tensor_tensor(out=m2, in0=pr, in1=m2, op=MAXOP)

                def p1_finish(it):
                    hp, nb = it // 2, it % 2
                    qa0, qa1, _, _ = qk[hp]
                    nsl = slice(nb * 512, (nb + 1) * 512)
                    m2 = macc.pop(it)
                    mall0 = spool.tile([128, 512], F32, tag="mall")
                    nc.gpsimd.partition_all_reduce(mall0, m2[:, 0, :], channels=128, reduce_op=RMAX)
                    nc.scalar.dma_start(out=r(qa0[64:65, nsl]), in_=r(mall0[0:1, :]))
                    mall1 = spool.tile([128, 512], F32, tag="mall")
                    nc.gpsimd.partition_all_reduce(mall1, m2[:, 1, :], channels=128, reduce_op=RMAX)
                    nc.scalar.dma_start(out=r(qa1[64:65, nsl]), in_=r(mall1[0:1, :]))

                def emit_av(it, h, j):
                    hp = it // 2
                    hh = 2 * hp + h
                    g, s4 = hh // 4, hh % 4
                    v4 = v4g[g]
                    zp = zps[(it, h)]
                    for t in range(2):
                        mt = 2 * j + t
                        et = expt.pop((it, h, mt))
                        nc.tensor.matmul(zp[0:65, :], v4[:, mt, s4, 0:65], et,
                                         start=(j == 0 and t == 0), stop=(j == 7 and t == 1))

                def p2av_step(it, step):
                    hp, nb = it // 2, it % 2
                    qa0, qa1, ka0, ka1 = qk[hp]
                    nsl = slice(nb * 512, (nb + 1) * 512)
                    h, j = step % 2, step // 2
                    if j == 0:
                        zps[(it, h)] = ps_z.tile([128, 512], F32, tag="zps", name="zps")
                    if j >= 1:
                        emit_av(it, h, j - 1)
                    for t in range(2):
                        mt = 2 * j + t
                        msl = slice(mt * 128, (mt + 1) * 128)
                        ka = ka0 if h == 0 else ka1
                        qa = qa0 if h == 0 else qa1
                        ps2t = ps_2.tile([128, 512], F32, tag="ps2", name="ps2t")
                        nc.tensor.matmul(ps2t, r(ka[0:65, msl]), r(qa[0:65, nsl]), start=True, stop=True)
                        et = epool.tile([128, 512], BF16, tag="expt", name="et")
                        nc.scalar.activation(out=et, in_=ps2t, func=EXP, scale=float(SCALE))
                        expt[(it, h, mt)] = et

                def p2av_finish(it):
                    hp, nb = it // 2, it % 2
                    nsl = slice(nb * 512, (nb + 1) * 512)
                    emit_av(it, 0, 7)
                    emit_av(it, 1, 7)
                    zp0 = zps.pop((it, 0))
                    zp1 = zps.pop((it, 1))
                    st = {}

                    def n0():
                        st["rec0"] = rec0 = spool.tile([65, 512], F32, tag="rec0", name="rec0")
                        with nc.allow_low_precision(reason="recip read as fp32r by PE broadcast"):
                            nc.vector.reciprocal(out=r(rec0[64:65, :]), in_=zp0[64:65, :])
                        st["rps0"] = rps0 = ps_2.tile([128, 512], F32, tag="ps2", name="rps0")
                        nc.tensor.matmul(rps0[0:65, :], r(ones65[64:65, 0:65]), r(rec0[64:65, :]), start=True, stop=True)

                    def n1():
                        st["recb0"] = recb0 = spool.tile([64, 512], F32, tag="recb0", name="recb0")
                        nc.scalar.copy(out=recb0, in_=st["rps0"][0:64, :])
                        nc.vector.tensor_tensor(out=r(zTn[0:64, hp, nsl]), in0=zp0[0:64, :], in1=recb0[0:64, :], op=MULT)

                    def n2():
                        st["rec1"] = rec1 = spool.tile([65, 512], F32, tag="rec1", name="rec1")
                        with nc.allow_low_precision(reason="recip read as fp32r by PE broadcast"):
                            nc.vector.reciprocal(out=r(rec1[64:65, :]), in_=zp1[64:65, :])
                        st["rps1"] = rps1 = ps_2.tile([128, 512], F32, tag="ps2", name="rps1")
                        nc.tensor.matmul(rps1[0:65, :], r(ones65[64:65, 0:65]), r(rec1[64:65, :]), start=True, stop=True)

                    def n3():
                        st["recb1"] = recb1 = spool.tile([64, 512], F32, tag="recb1", name="recb1")
                        nc.scalar.copy(out=recb1, in_=st["rps1"][0:64, :])
                        nc.vector.tensor_tensor(out=recb1[0:64, :], in0=zp1[0:64, :], in1=recb1[0:64, :], op=MULT)
                        nc.scalar.dma_start(out=r(zTn[64:128, hp, nsl]), in_=r(recb1[0:64, :]))

                    n0(); n2()
                    norm_q.extend([n1, n3])

                # hp0: q(0)/k(0)/k(1) inline (gated only on first x chunks), rest queued
                wv0 = queue_proj(0, immediate=0)
                pre = [projq.pop(0) for _ in range(6)]
                pre[0](); pre[1](); pre[3]()          # q0, k0, k1
                projq.extend([pre[2], pre[4], pre[5]])  # q1, k2, k3
                queue_vproj(0, wv0)

                for it in range(NIT + 2):
                    hp, nb = it // 2, it % 2
                    while norm_q:
                        norm_q.pop(0)()
                    if it < NIT - 1 and nb == 0:
                        prefetch_w(hp + 1)         # weights for next hp land during this it
                    if it < NIT - 1 and nb == 1:
                        wv_n = queue_proj(hp + 1)  # next hp's projections run in this body's steps
                        if wv_n is not None:
                            v4g["wv_pend"] = wv_n
                    if it < NIT and nb == 0 and hp > 0 and hp % 2 == 0:
                        queue_vproj(hp // 2, v4g.pop("wv_pend"))
                    prev = it - 2
                    for step in range(16):
                        if 0 <= prev:
                            p2av_step(prev, step)
                        if it < NIT:
                            p1_step(it, step)
                        if projq:
                            projq.pop(0)()
                    if 0 <= prev:
                        p2av_finish(prev)
                    if it < NIT:
                        p1_finish(it)
                while projq:
                    projq.pop(0)()
                while norm_q:
                    norm_q.pop(0)()
                # drop remaining refs so pools can close
                qk.clear(); v4g.clear(); wst.clear()

            # ---------------- phase 2: y^T = Wr bmm, then FF ----------------
            with tc.tile_pool(name="p2w", bufs=1) as p2w, \
                 tc.tile_pool(name="p2y", bufs=1) as p2y, \
                 tc.tile_pool(name="p2o", bufs=2) as p2o, \
                 tc.tile_pool(name="ps_y", bufs=2, space=PSUM) as ps_y:
                wr_sb = p2w.tile([128, 8, 8, 128], F32, tag="wr")   # [p, ec, dt, d]
                for dt in range(8):
                    nc.sync.dma_start(
                        out=r(wr_sb[:, :, dt, :]),
                        in_=r(wr_d.ap()).rearrange("(i p) c -> p i c", p=128)[:, :, dt * 128:(dt + 1) * 128])
                wff_sb = p2w.tile([128, 8, E], F32, tag="wff")
                for jc in range(8):
                    nc.sync.dma_start(out=r(wff_sb[:, :, jc * 128:(jc + 1) * 128]),
                                      in_=r(wfft_d.ap()).rearrange("(i p) c -> p i c", p=128)[:, :, jc * 128:(jc + 1) * 128])
                nc.sync.dma_start(out=bfft, in_=bff_d.ap().rearrange("(t p) o -> p (t o)", p=128))
                yT = p2y.tile([128, 8, NH], F32)
                for dt in range(8):
                    psy = ps_y.tile([128, NH], F32, tag="psy")
                    for ec in range(8):
                        for half in range(2):
                            nc.tensor.matmul(psy[:, half * 512:(half + 1) * 512],
                                             r(wr_sb[:, ec, dt, :]),
                                             r(zTn[:, ec, half * 512:(half + 1) * 512]),
                                             start=(ec == 0), stop=(ec == 7))
                    nc.scalar.copy(out=r(yT[:, dt, :]), in_=psy)
                for jt in range(8):
                    pso = ps_y.tile([128, NH], F32, tag="psy")
                    for dc in range(8):
                        for half in range(2):
                            nc.tensor.matmul(pso[:, half * 512:(half + 1) * 512],
                                             r(wff_sb[:, dc, jt * 128:(jt + 1) * 128]),
                                             r(yT[:, dc, half * 512:(half + 1) * 512]),
                                             start=(dc == 0), stop=(dc == 7))
                    ot = p2o.tile([128, NH], F32, tag="ot")
                    nc.scalar.activation(out=ot, in_=pso, func=RELU, bias=bfft[:, jt:jt + 1], scale=1.0)
                    nc.sync.dma_start(out=out_d.ap()[jt * 128:(jt + 1) * 128, :], in_=ot)

    nc.compile()
    return nc


def _get_module():
    if "nc" not in _CACHE:
        _CACHE["nc"] = _build()
    return _CACHE["nc"]


def kernel(x, attention_mask, Wq, Wk, Wv, Wr, Wff, bff):
    from concourse import bass_utils

    x = np.asarray(x, dtype=np.float32)
    attention_mask = np.asarray(attention_mask)
    Wq = np.asarray(Wq, dtype=np.float32)
    Wk = np.asarray(Wk, dtype=np.float32)
    Wv = np.asarray(Wv, dtype=np.float32)
    Wr = np.asarray(Wr, dtype=np.float32)
    Wff = np.asarray(Wff, dtype=np.float32)
    bff = np.asarray(bff, dtype=np.float32)

    if not np.all(attention_mask == 1):
        return _np_reference(x, attention_mask, Wq, Wk, Wv, Wr, Wff, bff)

    nc = _get_module()
    wq2 = np.ascontiguousarray(Wq.transpose(1, 0, 2).reshape(E, E))
    wk2 = np.ascontiguousarray(Wk.transpose(1, 0, 2).reshape(E, E))
    wv2 = np.ascontiguousarray(Wv.transpose(1, 0, 2).reshape(E, E))
    wfft = np.ascontiguousarray(Wff.T)
    bff2 = np.ascontiguousarray(bff.reshape(E, 1))

    in_maps = []
    for c in range(8):
        b, nh = c // 2, c % 2
        xt = x[b].T
        if nh:
            xt = np.concatenate([xt[:, NH:], xt[:, :NH]], axis=1)
        in_maps.append({
            "xt": np.ascontiguousarray(xt),
            "wq": wq2, "wk": wk2, "wv": wv2,
            "wr": np.ascontiguousarray(Wr[b]),
            "wfft": wfft, "bff": bff2,
            "cst": _CST,
        })

    res = bass_utils.run_bass_kernel_spmd(nc, in_maps, core_ids=list(range(8)), **_CACHE.get("run_kwargs", {}))
    _CACHE["last_result"] = res

    out = np.empty((B, S, E), dtype=np.float32)
    for c in range(8):
        b, nh = c // 2, c % 2
        out[b, nh * NH:(nh + 1) * NH, :] = res.results[c]["o"].T
    return out
